# revision 16
# baseline (speedup 1.0000x reference)
"""Trainium2 Bass kernel for a dense transformer block (B=4, N=1024, D=1024,
H=16, Dh=64, MLP 4x), distributed over 8 NeuronCores with ZERO collectives.

Sharding: core c handles batch b = c//2, sequence half = c%2 (512 query
rows).  K/V are computed for the batch's full 1024-token sequence on both
cores of a pair (the ~12% duplicated K/V FLOPs are far cheaper than the
~190us/16MB AllReduce the tensor-parallel split would need twice).  The
sequence is rotated per-core so the core's own 512 rows are always rows
0..511 of its input — attention is permutation-invariant over keys, so all
8 cores run one identical SPMD program.

Compute layout: residual stream stays natural [seq, d] in f32.  LN outputs
enter the transposed domain ([d, seq] bf16) via DMA-transpose bounced
through DRAM; Q^T/K^T/V and the MLP hidden G^T are produced transposed, and
the output projections (Wo, Wproj) consume the transposed activations as
the matmul's stationary operand, producing NATURAL-layout outputs whose
PSUM->SBUF copy is fused with the residual add.  Matmuls run in bf16 (PSUM
f32); softmax skips max-subtraction (scores ~N(0,0.4^2)) and normalizes
attention output after the AV matmul using a ones-column appended to V for
the denominators.
"""

import numpy as np

import bass_rust
import concourse.bass as bass
import concourse.mybir as mybir
import concourse.tile as tile

F32 = mybir.dt.float32
BF16 = mybir.dt.bfloat16
AF = mybir.ActivationFunctionType
ALU = mybir.AluOpType

P = 128
D = 1024
S = 1024          # full sequence (per batch)
SO = 512          # own rows per core
H = 16
DH = 64
F = 4096
EPS = 1e-5
N_CORES = 8

ND = D // P       # 8   d tiles
NS = S // P       # 8   full-seq tiles
NSO = SO // P     # 4   own-seq tiles
NF = F // P       # 32  ff tiles


# --------------------------------------------------------------------------
# Workaround: this compiler build supports only ONE semaphore wait per
# instruction.  Move excess waits onto fresh NOPs inserted just before the
# offending instruction on the same engine.
# --------------------------------------------------------------------------
_counter = [0]


def _split_multiwaits(nc):
    nsplit = 0
    for fn in nc.m.functions:
        for blk in fn.blocks:
            il = list(blk.instructions)
            out = []
            changed = False
            for inst in il:
                si = inst.sync_info
                if si is not None and len(si.on_wait) > 1:
                    waits = list(si.on_wait)
                    for w in waits[:-1]:
                        _counter[0] += 1
                        nop = mybir.InstNoOp(
                            name=f"I-waitsplit-{_counter[0]}", ins=[], outs=[]
                        )
                        nop.engine = inst.engine
                        nop.sync_info = bass_rust.SyncInfo(on_wait=[w], on_update=[])
                        out.append(nop)
                        nc.register_instruction(nop, overwrite=True)
                    inst.sync_info = bass_rust.SyncInfo(
                        on_wait=[waits[-1]], on_update=list(si.on_update)
                    )
                    changed = True
                    nsplit += 1
                out.append(inst)
            if changed:
                blk.instructions = out
    return nsplit


def _vec_tile(nc, pool, ext, n):
    """Load a [n*128] dram vector as a [128, n] sbuf tile (col i = tile i)."""
    t = pool.tile([P, n], F32, name=ext.name + "_sb")
    nc.sync.dma_start(out=t[:], in_=ext[:].rearrange("(o p) -> p o", p=P))
    return t


def _bcast_tile(nc, pool, ext, n):
    """Load a [n] dram vector broadcast to a [128, n] sbuf tile."""
    t = pool.tile([P, n], F32, name=ext.name + "_bc")
    ap = ext[:]
    src = bass.AP(tensor=ap.tensor, offset=ap.offset, ap=[[0, P], ap.ap[0]])
    nc.sync.dma_start(out=t[:], in_=src)
    return t


def build():
    nc = bass.Bass(name="tfblock")

    x_ext = nc.declare_dram_parameter("x", [S, D], F32, isOutput=False)
    ln1_w = nc.declare_dram_parameter("ln1_w", [D], F32, isOutput=False)
    ln1_b = nc.declare_dram_parameter("ln1_b", [D], F32, isOutput=False)
    Wq_e = nc.declare_dram_parameter("Wq", [D, D], F32, isOutput=False)
    bq_e = nc.declare_dram_parameter("bq", [D], F32, isOutput=False)
    Wk_e = nc.declare_dram_parameter("Wk", [D, D], F32, isOutput=False)
    bk_e = nc.declare_dram_parameter("bk", [D], F32, isOutput=False)
    Wv_e = nc.declare_dram_parameter("Wv", [D, D], F32, isOutput=False)
    bv_e = nc.declare_dram_parameter("bv", [D], F32, isOutput=False)
    Wo_e = nc.declare_dram_parameter("Wo", [D, D], F32, isOutput=False)
    bo_e = nc.declare_dram_parameter("bo", [D], F32, isOutput=False)
    ln2_w = nc.declare_dram_parameter("ln2_w", [D], F32, isOutput=False)
    ln2_b = nc.declare_dram_parameter("ln2_b", [D], F32, isOutput=False)
    Wfc_e = nc.declare_dram_parameter("Wfc", [D, F], F32, isOutput=False)
    bfc_e = nc.declare_dram_parameter("bfc", [F], F32, isOutput=False)
    Wp_e = nc.declare_dram_parameter("Wproj", [F, D], F32, isOutput=False)
    bp_e = nc.declare_dram_parameter("bproj", [D], F32, isOutput=False)
    out_ext = nc.declare_dram_parameter("out", [SO, D], F32, isOutput=True)

    cast_cycle = [0]

    def copy_cast(out, in_, eng=None):
        if eng is None:
            eng = ("v", "g", "s")[cast_cycle[0] % 3]
            cast_cycle[0] += 1
        e = {"v": 0, "g": 1, "s": 2}[eng]
        if e == 0:
            nc.vector.tensor_copy(out=out, in_=in_)
        elif e == 1:
            nc.gpsimd.tensor_copy(out=out, in_=in_)
        else:
            nc.scalar.copy(out=out, in_=in_)

    def ln_tile(lnp, src_ap, hn_out, eps_t, tag):
        """LayerNorm stats on DVE + apply on ACT: hn_out = (src-mu)*rstd."""
        stats = lnp.tile([P, 2, 6], F32, tag=tag + "_st")
        for g in range(2):
            nc.vector.bn_stats(out=stats[:, g, :], in_=src_ap[:, g * 512 : (g + 1) * 512])
        mv = lnp.tile([P, 2], F32, tag=tag + "_mv")
        nc.vector.bn_aggr(out=mv[:], in_=stats[:])
        std = lnp.tile([P, 1], F32, tag=tag + "_sd")
        nc.scalar.activation(out=std[:], in_=mv[:, 1:2], func=AF.Sqrt, bias=eps_t[:])
        rstd = lnp.tile([P, 1], F32, tag=tag + "_rs")
        nc.vector.reciprocal(out=rstd[:], in_=std[:])
        nb = lnp.tile([P, 1], F32, tag=tag + "_nb")
        nc.vector.tensor_scalar(nb[:], mv[:, 0:1], rstd[:], -1.0, ALU.mult, ALU.mult)
        nc.scalar.activation(
            out=hn_out, in_=src_ap, func=AF.Identity, bias=nb[:], scale=rstd[:]
        )

    with tile.TileContext(nc) as tc:
        from contextlib import ExitStack

        with ExitStack() as top:
            consts = top.enter_context(tc.tile_pool(name="consts", bufs=1))
            persist = top.enter_context(tc.tile_pool(name="persist", bufs=1))
            dram = top.enter_context(tc.tile_pool(name="dram", bufs=1, space="DRAM"))

            ln1w_t = _vec_tile(nc, consts, ln1_w, ND)
            ln1b_t = _vec_tile(nc, consts, ln1_b, ND)
            ln2w_t = _vec_tile(nc, consts, ln2_w, ND)
            ln2b_t = _vec_tile(nc, consts, ln2_b, ND)
            bq_t = _vec_tile(nc, consts, bq_e, ND)
            bk_t = _vec_tile(nc, consts, bk_e, ND)
            bfc_t = _vec_tile(nc, consts, bfc_e, NF)
            bv_bc = _bcast_tile(nc, consts, bv_e, D)
            bo_bc = _bcast_tile(nc, consts, bo_e, D)
            bp_bc = _bcast_tile(nc, consts, bp_e, D)

            eps_t = consts.tile([P, 1], F32, name="eps")
            nc.vector.memset(eps_t[:], EPS)
            ones64 = consts.tile([1, 64], F32, name="ones64")
            nc.vector.memset(ones64[:], 1.0)

            # xN_own lives until residual 1 (pre-biased with bo);
            # QT/KT/VN live until end of the Wo projection.
            xown_cm = tc.tile_pool(name="xown", bufs=1)
            xown = xown_cm.__enter__()
            xN_own = xown.tile([P, NSO, D], F32, name="xN_own")
            nc.sync.dma_start(
                out=xN_own[:], in_=x_ext[0:SO, :].rearrange("(t p) d -> p t d", p=P)
            )
            x1N = persist.tile([P, NSO, D], F32, name="x1N")

            qkv_cm = tc.tile_pool(name="qkvp", bufs=1)
            qkvp = qkv_cm.__enter__()

            # ---------------------------------- weight prefetch + LN1
            hN_dram = dram.tile([S, D], BF16, name="hN_dram")
            with ExitStack() as phB:
                wpool = phB.enter_context(tc.tile_pool(name="wqkv", bufs=1))
                stg = phB.enter_context(tc.tile_pool(name="stgB", bufs=3))
                psB = phB.enter_context(tc.tile_pool(name="psumB", bufs=2, space="PSUM"))
                hTp = phB.enter_context(tc.tile_pool(name="hTp", bufs=1))

                # kick off attention weight loads + casts first (gpsimd is
                # otherwise idle while DVE/ACT run LN1)
                Wq_bf = wpool.tile([P, ND, D], BF16, name="Wq_bf")
                Wk_bf = wpool.tile([P, ND, D], BF16, name="Wk_bf")
                Wv_bf = wpool.tile([P, ND, D], BF16, name="Wv_bf")
                for w_ext, w_bf in ((Wq_e, Wq_bf), (Wk_e, Wk_bf), (Wv_e, Wv_bf)):
                    for kt in range(ND):
                        s = stg.tile([P, D], F32, tag="wstg")
                        nc.sync.dma_start(out=s[:], in_=w_ext[kt * P : (kt + 1) * P, :])
                        copy_cast(w_bf[:, kt, :], s[:], eng="g")

                with tc.tile_pool(name="ln1", bufs=3) as lnp:
                    for st in range(NS):
                        xt = lnp.tile([P, D], F32, tag="xt")
                        nc.sync.dma_start(out=xt[:], in_=x_ext[st * P : (st + 1) * P, :])
                        hn = lnp.tile([P, D], BF16, tag="hn")
                        ln_tile(lnp, xt[:], hn[:], eps_t, "l1")
                        nc.sync.dma_start(out=hN_dram[st * P : (st + 1) * P, :], in_=hn[:])

                # h^T [d, s] bf16 (+ ln1 w/b on ACT), own half (s 0:512) first
                # so the Q projection can start before LN1's second half lands
                hT = hTp.tile([P, ND, S], BF16, name="hT")
                for sh in range(2):
                    for dt in range(ND):
                        nc.sync.dma_start_transpose(
                            hT[:, dt, sh * 512 : (sh + 1) * 512],
                            hN_dram[sh * 512 : (sh + 1) * 512, dt * P : (dt + 1) * P],
                        )
                        nc.scalar.activation(
                            out=hT[:, dt, sh * 512 : (sh + 1) * 512],
                            in_=hT[:, dt, sh * 512 : (sh + 1) * 512],
                            func=AF.Identity,
                            bias=ln1b_t[:, dt : dt + 1],
                            scale=ln1w_t[:, dt : dt + 1],
                        )

                QT = qkvp.tile([P, ND, SO], BF16, name="QT")
                KT = qkvp.tile([P, ND, S], BF16, name="KT")
                VN = qkvp.tile([P, NS, H, DH + 1], BF16, name="VN")
                nc.vector.memset(VN[:, :, :, DH : DH + 1], 1.0)

                for ot in range(ND):
                    ps = psB.tile([P, 512], F32, tag="ps_q")
                    for kt in range(ND):
                        nc.tensor.matmul(
                            ps[:],
                            Wq_bf[:, kt, ot * P : (ot + 1) * P],
                            hT[:, kt, 0:SO],
                            start=(kt == 0),
                            stop=(kt == ND - 1),
                        )
                    nc.vector.tensor_scalar(
                        QT[:, ot, :], ps[:], bq_t[:, ot : ot + 1], None, ALU.add
                    )
                for ot in range(ND):
                    for sh in range(2):
                        ps = psB.tile([P, 512], F32, tag="ps_k")
                        for kt in range(ND):
                            nc.tensor.matmul(
                                ps[:],
                                Wk_bf[:, kt, ot * P : (ot + 1) * P],
                                hT[:, kt, sh * 512 : (sh + 1) * 512],
                                start=(kt == 0),
                                stop=(kt == ND - 1),
                            )
                        nc.scalar.activation(
                            out=KT[:, ot, sh * 512 : (sh + 1) * 512],
                            in_=ps[:],
                            func=AF.Identity,
                            bias=bk_t[:, ot : ot + 1],
                        )
                for st in range(NS):
                    for oh in range(2):
                        ps = psB.tile([P, 512], F32, tag="ps_v")
                        for kt in range(ND):
                            nc.tensor.matmul(
                                ps[:],
                                hT[:, kt, st * P : (st + 1) * P],
                                Wv_bf[:, kt, oh * 512 : (oh + 1) * 512],
                                start=(kt == 0),
                                stop=(kt == ND - 1),
                            )
                        nc.vector.tensor_tensor(
                            VN[:, st, oh * 8 : (oh + 1) * 8, 0:DH],
                            ps[:].rearrange("p (h e) -> p h e", h=8),
                            bv_bc[:, oh * 512 : (oh + 1) * 512].rearrange(
                                "p (h e) -> p h e", h=8
                            ),
                            ALU.add,
                        )

            # ------------------------------------------------- attention
            with ExitStack() as phC:
                wo_pool = phC.enter_context(tc.tile_pool(name="wo", bufs=1))
                stgC = phC.enter_context(tc.tile_pool(name="stgC", bufs=3))
                otp = phC.enter_context(tc.tile_pool(name="otp", bufs=1))

                Wo_bf = wo_pool.tile([P, ND, D], BF16, name="Wo_bf")
                for kt in range(ND):
                    s = stgC.tile([P, D], F32, tag="wstgC")
                    nc.sync.dma_start(out=s[:], in_=Wo_e[kt * P : (kt + 1) * P, :])
                    copy_cast(Wo_bf[:, kt, :], s[:])

                # pre-bias the residual with bo (x + bo), in place
                for st in range(NSO):
                    nc.vector.tensor_tensor(
                        xN_own[:, st, :], xN_own[:, st, :], bo_bc[:], ALU.add
                    )

                OT = otp.tile([P, ND, SO], BF16, name="OT")

                phC1 = phC.enter_context(ExitStack())
                attn = phC1.enter_context(tc.tile_pool(name="attn", bufs=2))
                ps_s = phC1.enter_context(tc.tile_pool(name="ps_s", bufs=3, space="PSUM"))
                ps_o = phC1.enter_context(tc.tile_pool(name="ps_o", bufs=2, space="PSUM"))
                ps_bd = phC1.enter_context(tc.tile_pool(name="ps_bd", bufs=1, space="PSUM"))

                def normalize_pair(j, po_a, po_b):
                    # denominators live in psum row DH; broadcast 1/sum over
                    # the head's 64 partitions with a K=1 matmul
                    for off, po in ((0, po_a), (64, po_b)):
                        rec = attn.tile([1, SO], F32, tag="rec")
                        nc.vector.reciprocal(out=rec[:], in_=po[DH : DH + 1, :])
                        psb2 = ps_bd.tile([64, SO], F32, tag="ps_b")
                        nc.tensor.matmul(psb2[:], ones64[:], rec[:], start=True, stop=True)
                        bcast = attn.tile([64, SO], F32, tag="bcast")
                        nc.vector.tensor_copy(out=bcast[:], in_=psb2[:])
                        nc.vector.tensor_tensor(
                            OT[off : off + 64, j, :], po[0:DH, :], bcast[:], ALU.mult
                        )

                pending = None
                for j in range(H // 2):
                    pa = attn.tile([P, NS, SO], BF16, tag="probs_a")
                    pb = attn.tile([P, NS, SO], BF16, tag="probs_b")
                    po_a = ps_o.tile([P, SO], F32, tag="ps_oa")
                    po_b = ps_o.tile([P, SO], F32, tag="ps_ob")
                    for kb in range(NS):
                        psa = ps_s.tile([P, SO], F32, tag="ps_s")
                        psb = ps_s.tile([P, SO], F32, tag="ps_s")
                        nc.tensor.matmul(
                            psa[:],
                            KT[0:64, j, kb * P : (kb + 1) * P],
                            QT[0:64, j, :],
                            start=True,
                            stop=True,
                            tile_position=(0, 0),
                        )
                        nc.tensor.matmul(
                            psb[:],
                            KT[64:128, j, kb * P : (kb + 1) * P],
                            QT[64:128, j, :],
                            start=True,
                            stop=True,
                            tile_position=(64, 0),
                        )
                        nc.scalar.activation(
                            out=pa[:, kb, :], in_=psa[:], func=AF.Exp, scale=0.125
                        )
                        nc.scalar.activation(
                            out=pb[:, kb, :], in_=psb[:], func=AF.Exp, scale=0.125
                        )
                        nc.tensor.matmul(
                            po_a[0 : DH + 1, :],
                            VN[:, kb, 2 * j, :],
                            pa[:, kb, :],
                            start=(kb == 0),
                            stop=(kb == NS - 1),
                        )
                        nc.tensor.matmul(
                            po_b[0 : DH + 1, :],
                            VN[:, kb, 2 * j + 1, :],
                            pb[:, kb, :],
                            start=(kb == 0),
                            stop=(kb == NS - 1),
                        )
                    if pending is not None:
                        normalize_pair(*pending)
                    pending = (j, po_a, po_b)
                normalize_pair(*pending)
                phC1.close()

                # Wo projection, NATURAL output, fused residual:
                # x1[q, d] = (x + bo)[q, d] + sum_kt OT[:,kt,q].T @ Wo[kt, d]
                psD = phC.enter_context(tc.tile_pool(name="psD", bufs=2, space="PSUM"))
                for qb in range(NSO):
                    for dh in range(2):
                        ps = psD.tile([P, 512], F32, tag="ps_d")
                        for kt in range(ND):
                            nc.tensor.matmul(
                                ps[:],
                                OT[:, kt, qb * P : (qb + 1) * P],
                                Wo_bf[:, kt, dh * 512 : (dh + 1) * 512],
                                start=(kt == 0),
                                stop=(kt == ND - 1),
                            )
                        nc.vector.tensor_tensor(
                            x1N[:, qb, dh * 512 : (dh + 1) * 512],
                            xN_own[:, qb, dh * 512 : (dh + 1) * 512],
                            ps[:],
                            ALU.add,
                        )

            qkv_cm.__exit__(None, None, None)
            xown_cm.__exit__(None, None, None)

            # ------------------------------------------------------- LN2
            h2N_dram = dram.tile([SO, D], BF16, name="h2N_dram")
            with tc.tile_pool(name="ln2", bufs=3) as lnp:
                for st in range(NSO):
                    h2n = lnp.tile([P, D], BF16, tag="h2n")
                    ln_tile(lnp, x1N[:, st, :], h2n[:], eps_t, "l2")
                    nc.sync.dma_start(out=h2N_dram[st * P : (st + 1) * P, :], in_=h2n[:])

            # ------------------------------------------------------- MLP
            with ExitStack() as phF:
                h2p = phF.enter_context(tc.tile_pool(name="h2p", bufs=1))
                gtp = phF.enter_context(tc.tile_pool(name="gtp", bufs=1))
                wpp = phF.enter_context(tc.tile_pool(name="wpp", bufs=1))
                stgF = phF.enter_context(tc.tile_pool(name="stgF", bufs=4))
                wcst = phF.enter_context(tc.tile_pool(name="wcst", bufs=3))
                psF = phF.enter_context(tc.tile_pool(name="psF", bufs=2, space="PSUM"))
                opool = phF.enter_context(tc.tile_pool(name="opool", bufs=3))

                h2T = h2p.tile([P, ND, SO], BF16, name="h2T")
                for dt in range(ND):
                    nc.sync.dma_start_transpose(
                        h2T[:, dt, :], h2N_dram[:, dt * P : (dt + 1) * P]
                    )
                    nc.scalar.activation(
                        out=h2T[:, dt, :],
                        in_=h2T[:, dt, :],
                        func=AF.Identity,
                        bias=ln2b_t[:, dt : dt + 1],
                        scale=ln2w_t[:, dt : dt + 1],
                    )

                GT = gtp.tile([P, NF, SO], BF16, name="GT")
                Wp_bf = wpp.tile([P, NF, D], BF16, name="Wp_bf")

                for ft in range(NF):
                    # stream + cast Wfc column block (split DMAs for queue ||)
                    sfc = stgF.tile([P, ND, P], F32, tag="sfc")
                    for hh in range(2):
                        nc.sync.dma_start(
                            out=sfc[:, hh * 4 : (hh + 1) * 4, :],
                            in_=Wfc_e[
                                hh * 512 : (hh + 1) * 512, ft * P : (ft + 1) * P
                            ].rearrange("(kt p) f -> p kt f", p=P),
                        )
                    wfc_bf = wcst.tile([P, ND, P], BF16, tag="wfc_bf")
                    copy_cast(wfc_bf[:], sfc[:])
                    # stream + cast Wproj row block
                    sp = stgF.tile([P, D], F32, tag="sp")
                    for hh in range(2):
                        nc.sync.dma_start(
                            out=sp[:, hh * 512 : (hh + 1) * 512],
                            in_=Wp_e[
                                ft * P : (ft + 1) * P, hh * 512 : (hh + 1) * 512
                            ],
                        )
                    copy_cast(Wp_bf[:, ft, :], sp[:])

                    ps = psF.tile([P, SO], F32, tag="ps_g")
                    for kt in range(ND):
                        nc.tensor.matmul(
                            ps[:],
                            wfc_bf[:, kt, :],
                            h2T[:, kt, :],
                            start=(kt == 0),
                            stop=(kt == ND - 1),
                        )
                    nc.scalar.activation(
                        out=GT[:, ft, :],
                        in_=ps[:],
                        func=AF.Gelu,
                        bias=bfc_t[:, ft : ft + 1],
                    )

                # pre-bias the residual with bproj (x1 + bproj), in place
                for st in range(NSO):
                    nc.vector.tensor_tensor(
                        x1N[:, st, :], x1N[:, st, :], bp_bc[:], ALU.add
                    )

                # proj, NATURAL output, fused residual:
                # out[s, d] = (x1 + bproj)[s, d] + sum_ft GT[:,ft,s].T @ Wp[ft, d]
                for qb in range(NSO):
                    for dh in range(2):
                        ps = psF.tile([P, 512], F32, tag="ps_p")
                        for ft in range(NF):
                            nc.tensor.matmul(
                                ps[:],
                                GT[:, ft, qb * P : (qb + 1) * P],
                                Wp_bf[:, ft, dh * 512 : (dh + 1) * 512],
                                start=(ft == 0),
                                stop=(ft == NF - 1),
                            )
                        of = opool.tile([P, 512], F32, tag="of")
                        nc.vector.tensor_tensor(
                            of[:],
                            x1N[:, qb, dh * 512 : (dh + 1) * 512],
                            ps[:],
                            ALU.add,
                        )
                        nc.sync.dma_start(
                            out=out_ext[qb * P : (qb + 1) * P, dh * 512 : (dh + 1) * 512],
                            in_=of[:],
                        )

    _split_multiwaits(nc)
    return nc


_NC_CACHE = None


def _get_nc():
    global _NC_CACHE
    if _NC_CACHE is None:
        _NC_CACHE = build()
    return _NC_CACHE


def make_in_maps(inputs):
    """Shard FULL inputs into per-core input maps (own rows rotated first)."""
    x = np.asarray(inputs["x"], dtype=np.float32)
    names = [
        "ln1_w", "ln1_b", "Wq", "bq", "Wk", "bk", "Wv", "bv", "Wo", "bo",
        "ln2_w", "ln2_b", "Wfc", "bfc", "Wproj", "bproj",
    ]
    shared = {n: np.ascontiguousarray(np.asarray(inputs[n], dtype=np.float32))
              for n in names}
    in_maps = []
    for c in range(N_CORES):
        b, half = c // 2, c % 2
        xb = x[b]
        x_core = np.concatenate(
            [xb[half * SO : (half + 1) * SO], xb[(1 - half) * SO : (2 - half) * SO]],
            axis=0,
        )
        m = {"x": np.ascontiguousarray(x_core)}
        m.update(shared)
        in_maps.append(m)
    return in_maps


def kernel(**inputs) -> np.ndarray:
    from concourse.bass_utils import run_bass_kernel_spmd

    nc = _get_nc()
    in_maps = make_in_maps(inputs)
    res = run_bass_kernel_spmd(nc, in_maps, list(range(N_CORES)))
    B = 4
    out = np.empty((B, S, D), dtype=np.float32)
    for c in range(N_CORES):
        b, half = c // 2, c % 2
        out[b, half * SO : (half + 1) * SO] = res.results[c]["out"]
    return out


# revision 18
# speedup vs baseline: 1.1736x; 1.1736x over previous
"""Trainium2 Bass kernel for a dense transformer block (B=4, N=1024, D=1024,
H=16, Dh=64, MLP 4x), distributed over 8 NeuronCores with ZERO collectives.

Sharding: core c handles batch b = c//2, sequence half = c%2 (512 query
rows).  K/V are computed for the batch's full 1024-token sequence on both
cores of a pair (the ~12% duplicated K/V FLOPs are far cheaper than the
~190us/16MB AllReduce the tensor-parallel split would need twice).  The
sequence is rotated per-core so the core's own 512 rows are always rows
0..511 of its input — attention is permutation-invariant over keys, so all
8 cores run one identical SPMD program.

Compute layout: residual stream stays natural [seq, d] in f32.  LN outputs
enter the transposed domain ([d, seq] bf16) via DMA-transpose bounced
through DRAM; Q^T/K^T/V and the MLP hidden G^T are produced transposed, and
the output projections (Wo, Wproj) consume the transposed activations as
the matmul's stationary operand, producing NATURAL-layout outputs whose
PSUM->SBUF copy is fused with the residual add.  Matmuls run in bf16 (PSUM
f32); softmax skips max-subtraction (scores ~N(0,0.4^2)) and normalizes
attention output after the AV matmul using a ones-column appended to V for
the denominators.
"""

import numpy as np

import bass_rust
import concourse.bass as bass
import concourse.mybir as mybir
import concourse.tile as tile
from concourse.masks import make_identity

F32 = mybir.dt.float32
BF16 = mybir.dt.bfloat16
AF = mybir.ActivationFunctionType
ALU = mybir.AluOpType

P = 128
D = 1024
S = 1024          # full sequence (per batch)
SO = 512          # own rows per core
H = 16
DH = 64
F = 4096
EPS = 1e-5
N_CORES = 8

ND = D // P       # 8   d tiles
NS = S // P       # 8   full-seq tiles
NSO = SO // P     # 4   own-seq tiles
NF = F // P       # 32  ff tiles


# --------------------------------------------------------------------------
# Workaround: this compiler build supports only ONE semaphore wait per
# instruction.  Move excess waits onto fresh NOPs inserted just before the
# offending instruction on the same engine.
# --------------------------------------------------------------------------
_counter = [0]


def _split_multiwaits(nc):
    nsplit = 0
    for fn in nc.m.functions:
        for blk in fn.blocks:
            il = list(blk.instructions)
            out = []
            changed = False
            for inst in il:
                si = inst.sync_info
                if si is not None and len(si.on_wait) > 1:
                    waits = list(si.on_wait)
                    for w in waits[:-1]:
                        _counter[0] += 1
                        nop = mybir.InstNoOp(
                            name=f"I-waitsplit-{_counter[0]}", ins=[], outs=[]
                        )
                        nop.engine = inst.engine
                        nop.sync_info = bass_rust.SyncInfo(on_wait=[w], on_update=[])
                        out.append(nop)
                        nc.register_instruction(nop, overwrite=True)
                    inst.sync_info = bass_rust.SyncInfo(
                        on_wait=[waits[-1]], on_update=list(si.on_update)
                    )
                    changed = True
                    nsplit += 1
                out.append(inst)
            if changed:
                blk.instructions = out
    return nsplit


def _vec_tile(nc, pool, ext, n):
    """Load a [n*128] dram vector as a [128, n] sbuf tile (col i = tile i)."""
    t = pool.tile([P, n], F32, name=ext.name + "_sb")
    nc.sync.dma_start(out=t[:], in_=ext[:].rearrange("(o p) -> p o", p=P))
    return t


def _bcast_tile(nc, pool, ext, n):
    """Load a [n] dram vector broadcast to a [128, n] sbuf tile."""
    t = pool.tile([P, n], F32, name=ext.name + "_bc")
    ap = ext[:]
    src = bass.AP(tensor=ap.tensor, offset=ap.offset, ap=[[0, P], ap.ap[0]])
    nc.sync.dma_start(out=t[:], in_=src)
    return t


def build():
    nc = bass.Bass(name="tfblock")

    x_ext = nc.declare_dram_parameter("x", [S, D], F32, isOutput=False)
    ln1_w = nc.declare_dram_parameter("ln1_w", [D], F32, isOutput=False)
    ln1_b = nc.declare_dram_parameter("ln1_b", [D], F32, isOutput=False)
    Wq_e = nc.declare_dram_parameter("Wq", [D, D], F32, isOutput=False)
    bq_e = nc.declare_dram_parameter("bq", [D], F32, isOutput=False)
    Wk_e = nc.declare_dram_parameter("Wk", [D, D], F32, isOutput=False)
    bk_e = nc.declare_dram_parameter("bk", [D], F32, isOutput=False)
    Wv_e = nc.declare_dram_parameter("Wv", [D, D], F32, isOutput=False)
    bv_e = nc.declare_dram_parameter("bv", [D], F32, isOutput=False)
    Wo_e = nc.declare_dram_parameter("Wo", [D, D], F32, isOutput=False)
    bo_e = nc.declare_dram_parameter("bo", [D], F32, isOutput=False)
    ln2_w = nc.declare_dram_parameter("ln2_w", [D], F32, isOutput=False)
    ln2_b = nc.declare_dram_parameter("ln2_b", [D], F32, isOutput=False)
    Wfc_e = nc.declare_dram_parameter("Wfc", [D, F], F32, isOutput=False)
    bfc_e = nc.declare_dram_parameter("bfc", [F], F32, isOutput=False)
    Wp_e = nc.declare_dram_parameter("Wproj", [F, D], F32, isOutput=False)
    bp_e = nc.declare_dram_parameter("bproj", [D], F32, isOutput=False)
    out_ext = nc.declare_dram_parameter("out", [SO, D], F32, isOutput=True)

    cast_cycle = [0]

    def copy_cast(out, in_, eng=None):
        if eng is None:
            eng = ("v", "g", "s")[cast_cycle[0] % 3]
            cast_cycle[0] += 1
        e = {"v": 0, "g": 1, "s": 2}[eng]
        if e == 0:
            nc.vector.tensor_copy(out=out, in_=in_)
        elif e == 1:
            nc.gpsimd.tensor_copy(out=out, in_=in_)
        else:
            nc.scalar.copy(out=out, in_=in_)

    def ln_tile(lnp, src_ap, hn_out, eps_t, tag):
        """LayerNorm stats on DVE + apply on ACT: hn_out = (src-mu)*rstd."""
        stats = lnp.tile([P, 2, 6], F32, tag=tag + "_st")
        for g in range(2):
            nc.vector.bn_stats(out=stats[:, g, :], in_=src_ap[:, g * 512 : (g + 1) * 512])
        mv = lnp.tile([P, 2], F32, tag=tag + "_mv")
        nc.vector.bn_aggr(out=mv[:], in_=stats[:])
        std = lnp.tile([P, 1], F32, tag=tag + "_sd")
        nc.scalar.activation(out=std[:], in_=mv[:, 1:2], func=AF.Sqrt, bias=eps_t[:])
        rstd = lnp.tile([P, 1], F32, tag=tag + "_rs")
        nc.vector.reciprocal(out=rstd[:], in_=std[:])
        nb = lnp.tile([P, 1], F32, tag=tag + "_nb")
        nc.vector.tensor_scalar(nb[:], mv[:, 0:1], rstd[:], -1.0, ALU.mult, ALU.mult)
        nc.scalar.activation(
            out=hn_out, in_=src_ap, func=AF.Identity, bias=nb[:], scale=rstd[:]
        )

    with tile.TileContext(nc) as tc:
        from contextlib import ExitStack

        with ExitStack() as top:
            consts = top.enter_context(tc.tile_pool(name="consts", bufs=1))
            persist = top.enter_context(tc.tile_pool(name="persist", bufs=1))
            dram = top.enter_context(tc.tile_pool(name="dram", bufs=1, space="DRAM"))

            ln1w_t = _vec_tile(nc, consts, ln1_w, ND)
            ln1b_t = _vec_tile(nc, consts, ln1_b, ND)
            ln2w_t = _vec_tile(nc, consts, ln2_w, ND)
            ln2b_t = _vec_tile(nc, consts, ln2_b, ND)
            bq_t = _vec_tile(nc, consts, bq_e, ND)
            bk_t = _vec_tile(nc, consts, bk_e, ND)
            bfc_t = _vec_tile(nc, consts, bfc_e, NF)
            bv_bc = _bcast_tile(nc, consts, bv_e, D)
            bo_bc = _bcast_tile(nc, consts, bo_e, D)
            bp_bc = _bcast_tile(nc, consts, bp_e, D)

            eps_t = consts.tile([P, 1], F32, name="eps")
            nc.vector.memset(eps_t[:], EPS)
            ones64 = consts.tile([1, 64], F32, name="ones64")
            nc.vector.memset(ones64[:], 1.0)
            ident = consts.tile([P, P], BF16, name="ident")
            make_identity(nc, ident[:])

            # xN_own lives until residual 1 (pre-biased with bo);
            # QT/KT/VN live until end of the Wo projection.
            xown_cm = tc.tile_pool(name="xown", bufs=1)
            xown = xown_cm.__enter__()
            xN_own = xown.tile([P, NSO, D], F32, name="xN_own")
            nc.sync.dma_start(
                out=xN_own[:], in_=x_ext[0:SO, :].rearrange("(t p) d -> p t d", p=P)
            )
            x1N = persist.tile([P, NSO, D], F32, name="x1N")

            qkv_cm = tc.tile_pool(name="qkvp", bufs=1)
            qkvp = qkv_cm.__enter__()

            # ------------------------- LN1 (keeps hn in SBUF), weights, QKV
            with ExitStack() as phB:
                wpool = phB.enter_context(tc.tile_pool(name="wqkv", bufs=1))
                stg = phB.enter_context(tc.tile_pool(name="stgB", bufs=3))
                psB = phB.enter_context(tc.tile_pool(name="psumB", bufs=2, space="PSUM"))
                hTp = phB.enter_context(tc.tile_pool(name="hTp", bufs=1))

                hnN = hTp.tile([P, NS, D], BF16, name="hnN")
                with tc.tile_pool(name="ln1", bufs=3) as lnp:
                    for st in range(NS):
                        xt = lnp.tile([P, D], F32, tag="xt")
                        nc.sync.dma_start(out=xt[:], in_=x_ext[st * P : (st + 1) * P, :])
                        ln_tile(lnp, xt[:], hnN[:, st, :], eps_t, "l1")

                # h^T via PE-transpose, ln1 w/b fused into the ACT copy-back.
                # Own half (st 0..3) first so Q can start early.
                hT_own = hTp.tile([P, ND, SO], BF16, name="hT_own")
                hT_oth = hTp.tile([P, ND, SO], BF16, name="hT_oth")

                def transpose_half(hTx, sh):
                    for st4 in range(4):
                        st = sh * 4 + st4
                        for dt in range(ND):
                            pst = psB.tile([P, P], BF16, tag="ps_t")
                            nc.tensor.transpose(
                                pst[:], hnN[:, st, dt * P : (dt + 1) * P], ident[:]
                            )
                            nc.scalar.activation(
                                out=hTx[:, dt, st4 * P : (st4 + 1) * P],
                                in_=pst[:],
                                func=AF.Identity,
                                bias=ln1b_t[:, dt : dt + 1],
                                scale=ln1w_t[:, dt : dt + 1],
                            )

                transpose_half(hT_own, 0)

                Wq_bf = wpool.tile([P, ND, D], BF16, name="Wq_bf")
                Wk_bf = wpool.tile([P, ND, D], BF16, name="Wk_bf")
                Wv_bf = wpool.tile([P, ND, D], BF16, name="Wv_bf")
                for w_ext, w_bf, engs in (
                    (Wq_e, Wq_bf, ("v", "s")),
                    (Wk_e, Wk_bf, ("g",)),
                    (Wv_e, Wv_bf, ("v", "s")),
                ):
                    for kt in range(ND):
                        s = stg.tile([P, D], F32, tag="wstg")
                        nc.sync.dma_start(out=s[:], in_=w_ext[kt * P : (kt + 1) * P, :])
                        copy_cast(w_bf[:, kt, :], s[:], eng=engs[kt % len(engs)])

                QT = qkvp.tile([P, ND, SO], BF16, name="QT")
                KT = qkvp.tile([P, ND, S], BF16, name="KT")
                VN = qkvp.tile([P, NS, H, DH + 1], BF16, name="VN")
                nc.vector.memset(VN[:, :, :, DH : DH + 1], 1.0)

                for ot in range(ND):
                    ps = psB.tile([P, 512], F32, tag="ps_q")
                    for kt in range(ND):
                        nc.tensor.matmul(
                            ps[:],
                            Wq_bf[:, kt, ot * P : (ot + 1) * P],
                            hT_own[:, kt, :],
                            start=(kt == 0),
                            stop=(kt == ND - 1),
                        )
                    nc.vector.tensor_scalar(
                        QT[:, ot, :], ps[:], bq_t[:, ot : ot + 1], None, ALU.add
                    )

                transpose_half(hT_oth, 1)

                for ot in range(ND):
                    for sh in range(2):
                        hTx = hT_own if sh == 0 else hT_oth
                        ps = psB.tile([P, 512], F32, tag="ps_k")
                        for kt in range(ND):
                            nc.tensor.matmul(
                                ps[:],
                                Wk_bf[:, kt, ot * P : (ot + 1) * P],
                                hTx[:, kt, :],
                                start=(kt == 0),
                                stop=(kt == ND - 1),
                            )
                        nc.scalar.activation(
                            out=KT[:, ot, sh * 512 : (sh + 1) * 512],
                            in_=ps[:],
                            func=AF.Identity,
                            bias=bk_t[:, ot : ot + 1],
                        )
                for st in range(NS):
                    hTx = hT_own if st < 4 else hT_oth
                    st4 = st % 4
                    for oh in range(2):
                        ps = psB.tile([P, 512], F32, tag="ps_v")
                        for kt in range(ND):
                            nc.tensor.matmul(
                                ps[:],
                                hTx[:, kt, st4 * P : (st4 + 1) * P],
                                Wv_bf[:, kt, oh * 512 : (oh + 1) * 512],
                                start=(kt == 0),
                                stop=(kt == ND - 1),
                            )
                        nc.vector.tensor_tensor(
                            VN[:, st, oh * 8 : (oh + 1) * 8, 0:DH],
                            ps[:].rearrange("p (h e) -> p h e", h=8),
                            bv_bc[:, oh * 512 : (oh + 1) * 512].rearrange(
                                "p (h e) -> p h e", h=8
                            ),
                            ALU.add,
                        )

            # ------------------------------------------------- attention
            with ExitStack() as phC:
                wo_pool = phC.enter_context(tc.tile_pool(name="wo", bufs=1))
                stgC = phC.enter_context(tc.tile_pool(name="stgC", bufs=3))
                otp = phC.enter_context(tc.tile_pool(name="otp", bufs=1))

                Wo_bf = wo_pool.tile([P, ND, D], BF16, name="Wo_bf")
                for kt in range(ND):
                    s = stgC.tile([P, D], F32, tag="wstgC")
                    nc.sync.dma_start(out=s[:], in_=Wo_e[kt * P : (kt + 1) * P, :])
                    copy_cast(Wo_bf[:, kt, :], s[:], eng="g")

                # pre-bias the residual with bo (x + bo), in place
                for st in range(NSO):
                    nc.vector.tensor_tensor(
                        xN_own[:, st, :], xN_own[:, st, :], bo_bc[:], ALU.add
                    )

                OT = otp.tile([P, ND, SO], BF16, name="OT")

                phC1 = phC.enter_context(ExitStack())
                attn = phC1.enter_context(tc.tile_pool(name="attn", bufs=2))
                ps_s = phC1.enter_context(tc.tile_pool(name="ps_s", bufs=3, space="PSUM"))
                ps_o = phC1.enter_context(tc.tile_pool(name="ps_o", bufs=2, space="PSUM"))
                ps_bd = phC1.enter_context(tc.tile_pool(name="ps_bd", bufs=1, space="PSUM"))

                def normalize_pair(j, po_a, po_b):
                    # denominators live in psum row DH; broadcast 1/sum over
                    # the head's 64 partitions with a K=1 matmul
                    for off, po in ((0, po_a), (64, po_b)):
                        rec = attn.tile([1, SO], F32, tag="rec")
                        nc.vector.reciprocal(out=rec[:], in_=po[DH : DH + 1, :])
                        psb2 = ps_bd.tile([64, SO], F32, tag="ps_b")
                        nc.tensor.matmul(psb2[:], ones64[:], rec[:], start=True, stop=True)
                        bcast = attn.tile([64, SO], F32, tag="bcast")
                        nc.vector.tensor_copy(out=bcast[:], in_=psb2[:])
                        nc.vector.tensor_tensor(
                            OT[off : off + 64, j, :], po[0:DH, :], bcast[:], ALU.mult
                        )

                pending = None
                for j in range(H // 2):
                    pa = attn.tile([P, NS, SO], BF16, tag="probs_a")
                    pb = attn.tile([P, NS, SO], BF16, tag="probs_b")
                    po_a = ps_o.tile([P, SO], F32, tag="ps_oa")
                    po_b = ps_o.tile([P, SO], F32, tag="ps_ob")
                    for kb in range(NS):
                        psa = ps_s.tile([P, SO], F32, tag="ps_s")
                        psb = ps_s.tile([P, SO], F32, tag="ps_s")
                        nc.tensor.matmul(
                            psa[:],
                            KT[0:64, j, kb * P : (kb + 1) * P],
                            QT[0:64, j, :],
                            start=True,
                            stop=True,
                            tile_position=(0, 0),
                        )
                        nc.tensor.matmul(
                            psb[:],
                            KT[64:128, j, kb * P : (kb + 1) * P],
                            QT[64:128, j, :],
                            start=True,
                            stop=True,
                            tile_position=(64, 0),
                        )
                        nc.scalar.activation(
                            out=pa[:, kb, :], in_=psa[:], func=AF.Exp, scale=0.125
                        )
                        nc.scalar.activation(
                            out=pb[:, kb, :], in_=psb[:], func=AF.Exp, scale=0.125
                        )
                        nc.tensor.matmul(
                            po_a[0 : DH + 1, :],
                            VN[:, kb, 2 * j, :],
                            pa[:, kb, :],
                            start=(kb == 0),
                            stop=(kb == NS - 1),
                        )
                        nc.tensor.matmul(
                            po_b[0 : DH + 1, :],
                            VN[:, kb, 2 * j + 1, :],
                            pb[:, kb, :],
                            start=(kb == 0),
                            stop=(kb == NS - 1),
                        )
                    if pending is not None:
                        normalize_pair(*pending)
                    pending = (j, po_a, po_b)
                normalize_pair(*pending)
                phC1.close()

                # Wo projection, NATURAL output, fused residual:
                # x1[q, d] = (x + bo)[q, d] + sum_kt OT[:,kt,q].T @ Wo[kt, d]
                psD = phC.enter_context(tc.tile_pool(name="psD", bufs=2, space="PSUM"))
                for qb in range(NSO):
                    for dh in range(2):
                        ps = psD.tile([P, 512], F32, tag="ps_d")
                        for kt in range(ND):
                            nc.tensor.matmul(
                                ps[:],
                                OT[:, kt, qb * P : (qb + 1) * P],
                                Wo_bf[:, kt, dh * 512 : (dh + 1) * 512],
                                start=(kt == 0),
                                stop=(kt == ND - 1),
                            )
                        nc.vector.tensor_tensor(
                            x1N[:, qb, dh * 512 : (dh + 1) * 512],
                            xN_own[:, qb, dh * 512 : (dh + 1) * 512],
                            ps[:],
                            ALU.add,
                        )

            qkv_cm.__exit__(None, None, None)
            xown_cm.__exit__(None, None, None)

            # ----------------------------------------------- LN2 + MLP
            with ExitStack() as phF:
                h2p = phF.enter_context(tc.tile_pool(name="h2p", bufs=1))
                gtp = phF.enter_context(tc.tile_pool(name="gtp", bufs=1))
                wpp = phF.enter_context(tc.tile_pool(name="wpp", bufs=1))
                stgF = phF.enter_context(tc.tile_pool(name="stgF", bufs=4))
                wcst = phF.enter_context(tc.tile_pool(name="wcst", bufs=3))
                psF = phF.enter_context(tc.tile_pool(name="psF", bufs=2, space="PSUM"))
                opool = phF.enter_context(tc.tile_pool(name="opool", bufs=3))

                h2nN = h2p.tile([P, NSO, D], BF16, name="h2nN")
                with tc.tile_pool(name="ln2", bufs=3) as lnp:
                    for st in range(NSO):
                        ln_tile(lnp, x1N[:, st, :], h2nN[:, st, :], eps_t, "l2")

                # h2^T via PE-transpose, ln2 w/b fused into the ACT copy-back
                h2T = h2p.tile([P, ND, SO], BF16, name="h2T")
                for st in range(NSO):
                    for dt in range(ND):
                        pst = psF.tile([P, P], BF16, tag="ps_t2")
                        nc.tensor.transpose(
                            pst[:], h2nN[:, st, dt * P : (dt + 1) * P], ident[:]
                        )
                        nc.scalar.activation(
                            out=h2T[:, dt, st * P : (st + 1) * P],
                            in_=pst[:],
                            func=AF.Identity,
                            bias=ln2b_t[:, dt : dt + 1],
                            scale=ln2w_t[:, dt : dt + 1],
                        )

                GT = gtp.tile([P, NF, SO], BF16, name="GT")
                Wp_bf = wpp.tile([P, NF, D], BF16, name="Wp_bf")

                for ft in range(NF):
                    # stream + cast Wfc column block (split DMAs for queue ||)
                    sfc = stgF.tile([P, ND, P], F32, tag="sfc")
                    for hh in range(2):
                        nc.sync.dma_start(
                            out=sfc[:, hh * 4 : (hh + 1) * 4, :],
                            in_=Wfc_e[
                                hh * 512 : (hh + 1) * 512, ft * P : (ft + 1) * P
                            ].rearrange("(kt p) f -> p kt f", p=P),
                        )
                    wfc_bf = wcst.tile([P, ND, P], BF16, tag="wfc_bf")
                    copy_cast(wfc_bf[:], sfc[:])
                    # stream + cast Wproj row block
                    sp = stgF.tile([P, D], F32, tag="sp")
                    for hh in range(2):
                        nc.sync.dma_start(
                            out=sp[:, hh * 512 : (hh + 1) * 512],
                            in_=Wp_e[
                                ft * P : (ft + 1) * P, hh * 512 : (hh + 1) * 512
                            ],
                        )
                    copy_cast(Wp_bf[:, ft, :], sp[:])

                    ps = psF.tile([P, SO], F32, tag="ps_g")
                    for kt in range(ND):
                        nc.tensor.matmul(
                            ps[:],
                            wfc_bf[:, kt, :],
                            h2T[:, kt, :],
                            start=(kt == 0),
                            stop=(kt == ND - 1),
                        )
                    nc.scalar.activation(
                        out=GT[:, ft, :],
                        in_=ps[:],
                        func=AF.Gelu,
                        bias=bfc_t[:, ft : ft + 1],
                    )

                # pre-bias the residual with bproj (x1 + bproj), in place
                for st in range(NSO):
                    nc.vector.tensor_tensor(
                        x1N[:, st, :], x1N[:, st, :], bp_bc[:], ALU.add
                    )

                # proj, NATURAL output, fused residual:
                # out[s, d] = (x1 + bproj)[s, d] + sum_ft GT[:,ft,s].T @ Wp[ft, d]
                for qb in range(NSO):
                    for dh in range(2):
                        ps = psF.tile([P, 512], F32, tag="ps_p")
                        for ft in range(NF):
                            nc.tensor.matmul(
                                ps[:],
                                GT[:, ft, qb * P : (qb + 1) * P],
                                Wp_bf[:, ft, dh * 512 : (dh + 1) * 512],
                                start=(ft == 0),
                                stop=(ft == NF - 1),
                            )
                        of = opool.tile([P, 512], F32, tag="of")
                        nc.vector.tensor_tensor(
                            of[:],
                            x1N[:, qb, dh * 512 : (dh + 1) * 512],
                            ps[:],
                            ALU.add,
                        )
                        nc.sync.dma_start(
                            out=out_ext[qb * P : (qb + 1) * P, dh * 512 : (dh + 1) * 512],
                            in_=of[:],
                        )

    _split_multiwaits(nc)
    return nc


_NC_CACHE = None


def _get_nc():
    global _NC_CACHE
    if _NC_CACHE is None:
        _NC_CACHE = build()
    return _NC_CACHE


def make_in_maps(inputs):
    """Shard FULL inputs into per-core input maps (own rows rotated first)."""
    x = np.asarray(inputs["x"], dtype=np.float32)
    names = [
        "ln1_w", "ln1_b", "Wq", "bq", "Wk", "bk", "Wv", "bv", "Wo", "bo",
        "ln2_w", "ln2_b", "Wfc", "bfc", "Wproj", "bproj",
    ]
    shared = {n: np.ascontiguousarray(np.asarray(inputs[n], dtype=np.float32))
              for n in names}
    in_maps = []
    for c in range(N_CORES):
        b, half = c // 2, c % 2
        xb = x[b]
        x_core = np.concatenate(
            [xb[half * SO : (half + 1) * SO], xb[(1 - half) * SO : (2 - half) * SO]],
            axis=0,
        )
        m = {"x": np.ascontiguousarray(x_core)}
        m.update(shared)
        in_maps.append(m)
    return in_maps


def kernel(**inputs) -> np.ndarray:
    from concourse.bass_utils import run_bass_kernel_spmd

    nc = _get_nc()
    in_maps = make_in_maps(inputs)
    res = run_bass_kernel_spmd(nc, in_maps, list(range(N_CORES)))
    B = 4
    out = np.empty((B, S, D), dtype=np.float32)
    for c in range(N_CORES):
        b, half = c // 2, c % 2
        out[b, half * SO : (half + 1) * SO] = res.results[c]["out"]
    return out


# revision 25
# speedup vs baseline: 1.3920x; 1.1861x over previous
"""Trainium2 Bass kernel for a dense transformer block (B=4, N=1024, D=1024,
H=16, Dh=64, MLP 4x), distributed over 8 NeuronCores with ZERO collectives.

Sharding: core c handles batch b = c//2, sequence half = c%2 (512 query
rows).  K/V are computed for the batch's full 1024-token sequence on both
cores of a pair (the ~12% duplicated K/V FLOPs are far cheaper than the
~190us/16MB AllReduce the tensor-parallel split would need twice).  The
sequence is rotated per-core so the core's own 512 rows are always rows
0..511 of its input — attention is permutation-invariant over keys, so all
8 cores run one identical SPMD program.

Compute layout: residual stream stays natural [seq, d] in f32.  LN outputs
enter the transposed domain ([d, seq] bf16) via DMA-transpose bounced
through DRAM; Q^T/K^T/V and the MLP hidden G^T are produced transposed, and
the output projections (Wo, Wproj) consume the transposed activations as
the matmul's stationary operand, producing NATURAL-layout outputs whose
PSUM->SBUF copy is fused with the residual add.  Matmuls run in bf16 (PSUM
f32); softmax skips max-subtraction (scores ~N(0,0.4^2)) and normalizes
attention output after the AV matmul using a ones-column appended to V for
the denominators.
"""

import numpy as np

import bass_rust
import concourse.bass as bass
import concourse.mybir as mybir
import concourse.tile as tile
from concourse.masks import make_identity

F32 = mybir.dt.float32
BF16 = mybir.dt.bfloat16
AF = mybir.ActivationFunctionType
ALU = mybir.AluOpType

P = 128
D = 1024
S = 1024          # full sequence (per batch)
SO = 512          # own rows per core
H = 16
DH = 64
F = 4096
EPS = 1e-5
N_CORES = 8

ND = D // P       # 8   d tiles
NS = S // P       # 8   full-seq tiles
NSO = SO // P     # 4   own-seq tiles
NF = F // P       # 32  ff tiles


# --------------------------------------------------------------------------
# Workaround: this compiler build supports only ONE semaphore wait per
# instruction.  Move excess waits onto fresh NOPs inserted just before the
# offending instruction on the same engine.
# --------------------------------------------------------------------------
_counter = [0]


def _split_multiwaits(nc):
    nsplit = 0
    for fn in nc.m.functions:
        for blk in fn.blocks:
            il = list(blk.instructions)
            out = []
            changed = False
            for inst in il:
                si = inst.sync_info
                if si is not None and len(si.on_wait) > 1:
                    waits = list(si.on_wait)
                    for w in waits[:-1]:
                        _counter[0] += 1
                        nop = mybir.InstNoOp(
                            name=f"I-waitsplit-{_counter[0]}", ins=[], outs=[]
                        )
                        nop.engine = inst.engine
                        nop.sync_info = bass_rust.SyncInfo(on_wait=[w], on_update=[])
                        out.append(nop)
                        nc.register_instruction(nop, overwrite=True)
                    inst.sync_info = bass_rust.SyncInfo(
                        on_wait=[waits[-1]], on_update=list(si.on_update)
                    )
                    changed = True
                    nsplit += 1
                out.append(inst)
            if changed:
                blk.instructions = out
    return nsplit


def _vec_tile(nc, pool, ext, n):
    """Load a [n*128] dram vector as a [128, n] sbuf tile (col i = tile i)."""
    t = pool.tile([P, n], F32, name=ext.name + "_sb")
    nc.sync.dma_start(out=t[:], in_=ext[:].rearrange("(o p) -> p o", p=P))
    return t


def _bcast_tile(nc, pool, ext, n):
    """Load a [n] dram vector broadcast to a [128, n] sbuf tile."""
    t = pool.tile([P, n], F32, name=ext.name + "_bc")
    ap = ext[:]
    src = bass.AP(tensor=ap.tensor, offset=ap.offset, ap=[[0, P], ap.ap[0]])
    nc.sync.dma_start(out=t[:], in_=src)
    return t


def build():
    nc = bass.Bass(name="tfblock")

    x_ext = nc.declare_dram_parameter("x", [S, D], F32, isOutput=False)
    ln1_w = nc.declare_dram_parameter("ln1_w", [D], F32, isOutput=False)
    ln1_b = nc.declare_dram_parameter("ln1_b", [D], F32, isOutput=False)
    Wq_e = nc.declare_dram_parameter("Wq", [D, D], F32, isOutput=False)
    bq_e = nc.declare_dram_parameter("bq", [D], F32, isOutput=False)
    Wk_e = nc.declare_dram_parameter("Wk", [D, D], F32, isOutput=False)
    bk_e = nc.declare_dram_parameter("bk", [D], F32, isOutput=False)
    Wv_e = nc.declare_dram_parameter("Wv", [D, D], F32, isOutput=False)
    bv_e = nc.declare_dram_parameter("bv", [D], F32, isOutput=False)
    Wo_e = nc.declare_dram_parameter("Wo", [D, D], F32, isOutput=False)
    bo_e = nc.declare_dram_parameter("bo", [D], F32, isOutput=False)
    ln2_w = nc.declare_dram_parameter("ln2_w", [D], F32, isOutput=False)
    ln2_b = nc.declare_dram_parameter("ln2_b", [D], F32, isOutput=False)
    Wfc_e = nc.declare_dram_parameter("Wfc", [D, F], F32, isOutput=False)
    bfc_e = nc.declare_dram_parameter("bfc", [F], F32, isOutput=False)
    Wp_e = nc.declare_dram_parameter("Wproj", [F, D], F32, isOutput=False)
    bp_e = nc.declare_dram_parameter("bproj", [D], F32, isOutput=False)
    out_ext = nc.declare_dram_parameter("out", [SO, D], F32, isOutput=True)

    cast_cycle = [0]

    def copy_cast(out, in_, eng=None):
        if eng is None:
            eng = ("v", "g", "s")[cast_cycle[0] % 3]
            cast_cycle[0] += 1
        e = {"v": 0, "g": 1, "s": 2}[eng]
        if e == 0:
            nc.vector.tensor_copy(out=out, in_=in_)
        elif e == 1:
            nc.gpsimd.tensor_copy(out=out, in_=in_)
        else:
            nc.scalar.copy(out=out, in_=in_)

    def ln_tile(lnp, src_ap, hn_out, eps_t, tag):
        """LayerNorm stats on DVE + apply on ACT: hn_out = (src-mu)*rstd."""
        stats = lnp.tile([P, 2, 6], F32, tag=tag + "_st")
        for g in range(2):
            nc.vector.bn_stats(out=stats[:, g, :], in_=src_ap[:, g * 512 : (g + 1) * 512])
        mv = lnp.tile([P, 2], F32, tag=tag + "_mv")
        nc.vector.bn_aggr(out=mv[:], in_=stats[:])
        lnv = lnp.tile([P, 1], F32, tag=tag + "_sd")
        nc.scalar.activation(out=lnv[:], in_=mv[:, 1:2], func=AF.Ln, bias=eps_t[:])
        rstd = lnp.tile([P, 1], F32, tag=tag + "_rs")
        nc.scalar.activation(out=rstd[:], in_=lnv[:], func=AF.Exp, scale=-0.5)
        nb = lnp.tile([P, 1], F32, tag=tag + "_nb")
        nc.vector.tensor_scalar(nb[:], mv[:, 0:1], rstd[:], -1.0, ALU.mult, ALU.mult)
        nc.scalar.activation(
            out=hn_out, in_=src_ap, func=AF.Identity, bias=nb[:], scale=rstd[:]
        )

    with tile.TileContext(nc) as tc:
        from contextlib import ExitStack

        with ExitStack() as top:
            consts = top.enter_context(tc.tile_pool(name="consts", bufs=1))
            persist = top.enter_context(tc.tile_pool(name="persist", bufs=1))
            dram = top.enter_context(tc.tile_pool(name="dram", bufs=1, space="DRAM"))

            ln1w_t = _vec_tile(nc, consts, ln1_w, ND)
            ln1b_t = _vec_tile(nc, consts, ln1_b, ND)
            ln2w_t = _vec_tile(nc, consts, ln2_w, ND)
            ln2b_t = _vec_tile(nc, consts, ln2_b, ND)
            bq_t = _vec_tile(nc, consts, bq_e, ND)
            bk_t = _vec_tile(nc, consts, bk_e, ND)
            bfc_t = _vec_tile(nc, consts, bfc_e, NF)
            bv_bc = _bcast_tile(nc, consts, bv_e, D)

            eps_t = consts.tile([P, 1], F32, name="eps")
            nc.vector.memset(eps_t[:], EPS)
            e0 = consts.tile([P, P], F32, name="e0")
            nc.vector.memset(e0[:], 0.0)
            nc.vector.memset(e0[0:1, :], 1.0)
            ident = consts.tile([P, P], BF16, name="ident")
            make_identity(nc, ident[:])

            # xN_own lives until residual 1 (pre-biased with bo);
            # QT/KT/VN live until end of the Wo projection.
            xown_cm = tc.tile_pool(name="xown", bufs=1)
            xown = xown_cm.__enter__()
            xN_own = xown.tile([P, NSO, D], F32, name="xN_own")
            nc.sync.dma_start(
                out=xN_own[:], in_=x_ext[0:SO, :].rearrange("(t p) d -> p t d", p=P)
            )
            x1N = persist.tile([P, NSO, D], F32, name="x1N")

            qkv_cm = tc.tile_pool(name="qkvp", bufs=1)
            qkvp = qkv_cm.__enter__()

            # ------------------------- LN1 (keeps hn in SBUF), weights, QKV
            with ExitStack() as phB:
                wpool = phB.enter_context(tc.tile_pool(name="wqkv", bufs=1))
                stg = phB.enter_context(tc.tile_pool(name="stgB", bufs=2))
                psB = phB.enter_context(tc.tile_pool(name="psumB", bufs=2, space="PSUM"))
                hTp = phB.enter_context(tc.tile_pool(name="hTp", bufs=1))

                hnN = hTp.tile([P, NS, D], BF16, name="hnN")
                with tc.tile_pool(name="ln1", bufs=2) as lnp:
                    for st in range(NS):
                        xt = lnp.tile([P, D], F32, tag="xt")
                        nc.sync.dma_start(out=xt[:], in_=x_ext[st * P : (st + 1) * P, :])
                        ln_tile(lnp, xt[:], hnN[:, st, :], eps_t, "l1")

                # h^T via PE-transpose, ln1 w/b fused into the ACT copy-back.
                # Own half (st 0..3) first so Q can start early.
                hT_own = hTp.tile([P, ND, SO], BF16, name="hT_own")
                hT_oth = hTp.tile([P, ND, SO], BF16, name="hT_oth")

                def transpose_half(hTx, sh):
                    for st4 in range(4):
                        st = sh * 4 + st4
                        for dt in range(ND):
                            pst = psB.tile([P, P], BF16, tag="ps_t")
                            nc.tensor.transpose(
                                pst[:], hnN[:, st, dt * P : (dt + 1) * P], ident[:]
                            )
                            nc.vector.tensor_scalar(
                                hTx[:, dt, st4 * P : (st4 + 1) * P],
                                pst[:],
                                ln1w_t[:, dt : dt + 1],
                                ln1b_t[:, dt : dt + 1],
                                ALU.mult,
                                ALU.add,
                            )

                transpose_half(hT_own, 0)

                Wq_bf = wpool.tile([P, ND, D], BF16, name="Wq_bf")
                Wk_bf = wpool.tile([P, ND, D], BF16, name="Wk_bf")
                Wv_bf = wpool.tile([P, ND, D], BF16, name="Wv_bf")
                for w_ext, w_bf, engs in (
                    (Wq_e, Wq_bf, ("v", "s")),
                    (Wk_e, Wk_bf, ("g",)),
                    (Wv_e, Wv_bf, ("v", "s")),
                ):
                    for kt in range(ND):
                        s = stg.tile([P, D], F32, tag="wstg")
                        nc.sync.dma_start(out=s[:], in_=w_ext[kt * P : (kt + 1) * P, :])
                        copy_cast(w_bf[:, kt, :], s[:], eng=engs[kt % len(engs)])

                QT = qkvp.tile([P, ND, SO], BF16, name="QT")
                KTe = qkvp.tile([P, ND, S], BF16, name="KTe")
                KTo = qkvp.tile([P, ND, S], BF16, name="KTo")
                VN = qkvp.tile([P, NS, H, P], BF16, name="VN")
                nc.gpsimd.memset(KTe[64:128, :, :], 0.0)
                nc.gpsimd.memset(KTo[0:64, :, :], 0.0)
                nc.vector.memset(VN[:, :, :, DH + 1 :], 0.0)
                nc.vector.memset(VN[:, :, :, DH : DH + 1], 1.0)

                for ot in range(ND):
                    ps = psB.tile([P, 512], F32, tag="ps_q")
                    for kt in range(ND):
                        nc.tensor.matmul(
                            ps[:],
                            Wq_bf[:, kt, ot * P : (ot + 1) * P],
                            hT_own[:, kt, :],
                            start=(kt == 0),
                            stop=(kt == ND - 1),
                        )
                    nc.vector.tensor_scalar(
                        QT[:, ot, :], ps[:], bq_t[:, ot : ot + 1], None, ALU.add
                    )

                transpose_half(hT_oth, 1)

                for ot in range(ND):
                    for sh in range(2):
                        hTx = hT_own if sh == 0 else hT_oth
                        ps = psB.tile([P, 512], F32, tag="ps_k")
                        for kt in range(ND):
                            nc.tensor.matmul(
                                ps[:],
                                Wk_bf[:, kt, ot * P : (ot + 1) * P],
                                hTx[:, kt, :],
                                start=(kt == 0),
                                stop=(kt == ND - 1),
                            )
                        nc.scalar.activation(
                            out=KTe[0:64, ot, sh * 512 : (sh + 1) * 512],
                            in_=ps[0:64, :],
                            func=AF.Identity,
                            bias=bk_t[0:64, ot : ot + 1],
                        )
                        nc.vector.tensor_scalar(
                            KTo[64:128, ot, sh * 512 : (sh + 1) * 512],
                            ps[64:128, :],
                            bk_t[64:128, ot : ot + 1],
                            None,
                            ALU.add,
                        )
                for st in range(NS):
                    hTx = hT_own if st < 4 else hT_oth
                    st4 = st % 4
                    for oh in range(2):
                        ps = psB.tile([P, 512], F32, tag="ps_v")
                        for kt in range(ND):
                            nc.tensor.matmul(
                                ps[:],
                                hTx[:, kt, st4 * P : (st4 + 1) * P],
                                Wv_bf[:, kt, oh * 512 : (oh + 1) * 512],
                                start=(kt == 0),
                                stop=(kt == ND - 1),
                            )
                        nc.vector.tensor_tensor(
                            VN[:, st, oh * 8 : (oh + 1) * 8, 0:DH],
                            ps[:].rearrange("p (h e) -> p h e", h=8),
                            bv_bc[:, oh * 512 : (oh + 1) * 512].rearrange(
                                "p (h e) -> p h e", h=8
                            ),
                            ALU.add,
                        )

            # ------------------------------------------------- attention
            with ExitStack() as phC:
                wo_pool = phC.enter_context(tc.tile_pool(name="wo", bufs=1))
                stgC = phC.enter_context(tc.tile_pool(name="stgC", bufs=3))
                otp = phC.enter_context(tc.tile_pool(name="otp", bufs=1))

                bo_bc = _bcast_tile(nc, wo_pool, bo_e, D)
                Wo_bf = wo_pool.tile([P, ND, D], BF16, name="Wo_bf")
                for kt in range(ND):
                    s = stgC.tile([P, D], F32, tag="wstgC")
                    nc.sync.dma_start(out=s[:], in_=Wo_e[kt * P : (kt + 1) * P, :])
                    copy_cast(Wo_bf[:, kt, :], s[:], eng="g")

                # pre-bias the residual with bo (x + bo), in place
                for st in range(NSO):
                    nc.vector.tensor_tensor(
                        xN_own[:, st, :], xN_own[:, st, :], bo_bc[:], ALU.add
                    )

                OT = otp.tile([P, ND, SO], BF16, name="OT")

                phC1 = phC.enter_context(ExitStack())
                attn = phC1.enter_context(tc.tile_pool(name="attn", bufs=2))
                ps_s = phC1.enter_context(tc.tile_pool(name="ps_s", bufs=3, space="PSUM"))
                ps_o = phC1.enter_context(tc.tile_pool(name="ps_o", bufs=2, space="PSUM"))
                ps_bd = phC1.enter_context(tc.tile_pool(name="ps_bd", bufs=1, space="PSUM"))

                def normalize_pair(j, po_a, po_b):
                    # denominators live in psum row DH; broadcast 1/sum over
                    # all partitions with a zero-padded K=128 matmul vs e0
                    for off, po in ((0, po_a), (64, po_b)):
                        rec = attn.tile([P, SO], F32, tag="rec")
                        nc.gpsimd.memset(rec[:], 0.0)
                        lnrow = attn.tile([1, SO], F32, tag="lnrow")
                        nc.scalar.activation(
                            out=lnrow[:], in_=po[DH : DH + 1, :], func=AF.Ln
                        )
                        nc.scalar.activation(
                            out=rec[0:1, :], in_=lnrow[:], func=AF.Exp, scale=-1.0
                        )
                        psb2 = ps_bd.tile([P, SO], F32, tag="ps_b")
                        nc.tensor.matmul(psb2[:], e0[:], rec[:], start=True, stop=True)
                        bcast = attn.tile([64, SO], F32, tag="bcast")
                        nc.vector.tensor_copy(out=bcast[:], in_=psb2[0:64, :])
                        nc.vector.tensor_tensor(
                            OT[off : off + 64, j, :], po[0:DH, :], bcast[:], ALU.mult
                        )

                pending = None
                for j in range(H // 2):
                    pa = attn.tile([P, NS, SO], BF16, tag="probs_a")
                    pb = attn.tile([P, NS, SO], BF16, tag="probs_b")
                    po_a = ps_o.tile([P, SO], F32, tag="ps_oa")
                    po_b = ps_o.tile([P, SO], F32, tag="ps_ob")
                    for kb in range(NS):
                        psa = ps_s.tile([P, SO], F32, tag="ps_s")
                        psb = ps_s.tile([P, SO], F32, tag="ps_s")
                        nc.tensor.matmul(
                            psa[:],
                            KTe[:, j, kb * P : (kb + 1) * P],
                            QT[:, j, :],
                            start=True,
                            stop=True,
                        )
                        nc.tensor.matmul(
                            psb[:],
                            KTo[:, j, kb * P : (kb + 1) * P],
                            QT[:, j, :],
                            start=True,
                            stop=True,
                        )
                        nc.scalar.activation(
                            out=pa[:, kb, :], in_=psa[:], func=AF.Exp, scale=0.125
                        )
                        nc.scalar.activation(
                            out=pb[:, kb, :], in_=psb[:], func=AF.Exp, scale=0.125
                        )
                        nc.tensor.matmul(
                            po_a[:],
                            VN[:, kb, 2 * j, :],
                            pa[:, kb, :],
                            start=(kb == 0),
                            stop=(kb == NS - 1),
                        )
                        nc.tensor.matmul(
                            po_b[:],
                            VN[:, kb, 2 * j + 1, :],
                            pb[:, kb, :],
                            start=(kb == 0),
                            stop=(kb == NS - 1),
                        )
                    if pending is not None:
                        normalize_pair(*pending)
                    pending = (j, po_a, po_b)
                normalize_pair(*pending)
                phC1.close()

                # Wo projection, NATURAL output, fused residual:
                # x1[q, d] = (x + bo)[q, d] + sum_kt OT[:,kt,q].T @ Wo[kt, d]
                psD = phC.enter_context(tc.tile_pool(name="psD", bufs=2, space="PSUM"))
                for qb in range(NSO):
                    for dh in range(2):
                        ps = psD.tile([P, 512], F32, tag="ps_d")
                        for kt in range(ND):
                            nc.tensor.matmul(
                                ps[:],
                                OT[:, kt, qb * P : (qb + 1) * P],
                                Wo_bf[:, kt, dh * 512 : (dh + 1) * 512],
                                start=(kt == 0),
                                stop=(kt == ND - 1),
                            )
                        nc.vector.tensor_tensor(
                            x1N[:, qb, dh * 512 : (dh + 1) * 512],
                            xN_own[:, qb, dh * 512 : (dh + 1) * 512],
                            ps[:],
                            ALU.add,
                        )

            qkv_cm.__exit__(None, None, None)
            xown_cm.__exit__(None, None, None)

            # ----------------------------------------------- LN2 + MLP
            with ExitStack() as phF:
                h2p = phF.enter_context(tc.tile_pool(name="h2p", bufs=1))
                gtp = phF.enter_context(tc.tile_pool(name="gtp", bufs=1))
                wpp = phF.enter_context(tc.tile_pool(name="wpp", bufs=1))
                stgF = phF.enter_context(tc.tile_pool(name="stgF", bufs=4))
                wcst = phF.enter_context(tc.tile_pool(name="wcst", bufs=3))
                psF = phF.enter_context(tc.tile_pool(name="psF", bufs=2, space="PSUM"))
                opool = phF.enter_context(tc.tile_pool(name="opool", bufs=3))

                bp_bc = _bcast_tile(nc, h2p, bp_e, D)
                h2nN = h2p.tile([P, NSO, D], BF16, name="h2nN")
                with tc.tile_pool(name="ln2", bufs=3) as lnp:
                    for st in range(NSO):
                        ln_tile(lnp, x1N[:, st, :], h2nN[:, st, :], eps_t, "l2")

                # h2^T via PE-transpose, ln2 w/b fused into the ACT copy-back
                h2T = h2p.tile([P, ND, SO], BF16, name="h2T")
                for st in range(NSO):
                    for dt in range(ND):
                        pst = psF.tile([P, P], BF16, tag="ps_t2")
                        nc.tensor.transpose(
                            pst[:], h2nN[:, st, dt * P : (dt + 1) * P], ident[:]
                        )
                        nc.vector.tensor_scalar(
                            h2T[:, dt, st * P : (st + 1) * P],
                            pst[:],
                            ln2w_t[:, dt : dt + 1],
                            ln2b_t[:, dt : dt + 1],
                            ALU.mult,
                            ALU.add,
                        )

                GT = gtp.tile([P, NF, SO], BF16, name="GT")
                Wp_bf = wpp.tile([P, NF, D], BF16, name="Wp_bf")

                for ft in range(NF):
                    # stream + cast Wfc column block (split DMAs for queue ||)
                    sfc = stgF.tile([P, ND, P], F32, tag="sfc")
                    for hh in range(2):
                        nc.sync.dma_start(
                            out=sfc[:, hh * 4 : (hh + 1) * 4, :],
                            in_=Wfc_e[
                                hh * 512 : (hh + 1) * 512, ft * P : (ft + 1) * P
                            ].rearrange("(kt p) f -> p kt f", p=P),
                        )
                    wfc_bf = wcst.tile([P, ND, P], BF16, tag="wfc_bf")
                    copy_cast(wfc_bf[:], sfc[:])
                    # stream + cast Wproj row block
                    sp = stgF.tile([P, D], F32, tag="sp")
                    for hh in range(2):
                        nc.sync.dma_start(
                            out=sp[:, hh * 512 : (hh + 1) * 512],
                            in_=Wp_e[
                                ft * P : (ft + 1) * P, hh * 512 : (hh + 1) * 512
                            ],
                        )
                    copy_cast(Wp_bf[:, ft, :], sp[:])

                    ps = psF.tile([P, SO], F32, tag="ps_g")
                    for kt in range(ND):
                        nc.tensor.matmul(
                            ps[:],
                            wfc_bf[:, kt, :],
                            h2T[:, kt, :],
                            start=(kt == 0),
                            stop=(kt == ND - 1),
                        )
                    nc.scalar.activation(
                        out=GT[:, ft, :],
                        in_=ps[:],
                        func=AF.Gelu,
                        bias=bfc_t[:, ft : ft + 1],
                    )

                # pre-bias the residual with bproj (x1 + bproj), in place
                for st in range(NSO):
                    nc.vector.tensor_tensor(
                        x1N[:, st, :], x1N[:, st, :], bp_bc[:], ALU.add
                    )

                # proj, NATURAL output, fused residual:
                # out[s, d] = (x1 + bproj)[s, d] + sum_ft GT[:,ft,s].T @ Wp[ft, d]
                for qb in range(NSO):
                    for dh in range(2):
                        ps = psF.tile([P, 512], F32, tag="ps_p")
                        for ft in range(NF):
                            nc.tensor.matmul(
                                ps[:],
                                GT[:, ft, qb * P : (qb + 1) * P],
                                Wp_bf[:, ft, dh * 512 : (dh + 1) * 512],
                                start=(ft == 0),
                                stop=(ft == NF - 1),
                            )
                        of = opool.tile([P, 512], F32, tag="of")
                        nc.vector.tensor_tensor(
                            of[:],
                            x1N[:, qb, dh * 512 : (dh + 1) * 512],
                            ps[:],
                            ALU.add,
                        )
                        nc.sync.dma_start(
                            out=out_ext[qb * P : (qb + 1) * P, dh * 512 : (dh + 1) * 512],
                            in_=of[:],
                        )

    _split_multiwaits(nc)
    return nc


_NC_CACHE = None


def _get_nc():
    global _NC_CACHE
    if _NC_CACHE is None:
        _NC_CACHE = build()
    return _NC_CACHE


def make_in_maps(inputs):
    """Shard FULL inputs into per-core input maps (own rows rotated first)."""
    x = np.asarray(inputs["x"], dtype=np.float32)
    names = [
        "ln1_w", "ln1_b", "Wq", "bq", "Wk", "bk", "Wv", "bv", "Wo", "bo",
        "ln2_w", "ln2_b", "Wfc", "bfc", "Wproj", "bproj",
    ]
    shared = {n: np.ascontiguousarray(np.asarray(inputs[n], dtype=np.float32))
              for n in names}
    in_maps = []
    for c in range(N_CORES):
        b, half = c // 2, c % 2
        xb = x[b]
        x_core = np.concatenate(
            [xb[half * SO : (half + 1) * SO], xb[(1 - half) * SO : (2 - half) * SO]],
            axis=0,
        )
        m = {"x": np.ascontiguousarray(x_core)}
        m.update(shared)
        in_maps.append(m)
    return in_maps


def kernel(**inputs) -> np.ndarray:
    from concourse.bass_utils import run_bass_kernel_spmd

    nc = _get_nc()
    in_maps = make_in_maps(inputs)
    res = run_bass_kernel_spmd(nc, in_maps, list(range(N_CORES)))
    B = 4
    out = np.empty((B, S, D), dtype=np.float32)
    for c in range(N_CORES):
        b, half = c // 2, c % 2
        out[b, half * SO : (half + 1) * SO] = res.results[c]["out"]
    return out
